# revision 24
# baseline (speedup 1.0000x reference)
"""Causal self-attention (B=4, T=2048, C=1024, H=16) on 8 TRN2 NeuronCores.

Sharding: core = (batch, head-group) on a 4x2 grid. Each core computes the
attention output of 8 heads for one batch element plus its partial out-proj
(y^T = w_out_slice^T @ out_heads^T); the two head-groups of a batch are summed
on the host (the "out_proj all-reduce"), where the final bias is also added.

On-chip dataflow is fully transposed so no transposes are ever needed:
  qk^T  = w_qkv_slice^T @ x^T          (C on partitions)
  v     = x @ w_v_slice                (T on partitions, natural)
  S^T   = k_h @ q_h^T                  (k-positions on partitions; the two
                                        heads of a pair run CONCURRENTLY as
                                        64-row PE tiles - q/k of head 0 live
                                        in partitions 0-63, head 1 in 64-127)
  P^T   = exp(S^T) * causal_mask       (no max-subtraction: scores ~ N(0,1);
                                        mask multiplies run on GpSimd)
  outT  = [v|1]^T @ P^T                (ones column accumulates sum-of-exp;
                                        the v block of a pair is laid out
                                        [v0|1|v1] so head 0 uses window
                                        [0:128] (out rows 0-63 + se at 64)
                                        and head 1 uses window [1:129]
                                        (se at 63... row 64-127 = out1) -
                                        both heads' outputs land on their own
                                        partitions, so the normalized result
                                        is written straight into OT by DVE)
  y^T   = w_out_slice^T @ (outT/sumexp)

Causal trimming: for a diagonal key block at offset d (0-3) only query
columns >= 128*d can attend it, so score matmuls / exp / mask / PV all
operate on the trailing 512-128*d columns only.

The PE queue is statically interleaved: projection matmul groups (v, qk of
the next pair, out-proj once a query chunk is fully normalized) are drained
one per score/PV iteration so the PE never idles while ACT works through the
exp stream, and the HAM clock gate never re-throttles.
"""

import sys
import types

if "/opt/trn_rl_repo" not in sys.path:
    sys.path.insert(0, "/opt/trn_rl_repo")

import numpy as np


def _install_ntff_hook_shim():
    """antenv.axon_hooks is missing in this image; provide it so that
    run_bass_kernel_spmd(trace=True) can capture NTFF profiles."""
    if "antenv.axon_hooks" in sys.modules:
        return
    try:
        from trn_agent_boot.trn_boot import _ntff_profile_via_ctypes

        hook = _ntff_profile_via_ctypes("/opt/axon/libaxon_pjrt.so")
    except Exception:
        hook = None
    m = types.ModuleType("antenv.axon_hooks")
    m.get_axon_ntff_profile_hook = lambda: hook
    sys.modules["antenv.axon_hooks"] = m


_install_ntff_hook_shim()

import concourse.bass as bass  # noqa: E402
from concourse import bacc  # noqa: E402
import concourse.mybir as mybir  # noqa: E402
import concourse.tile as tile  # noqa: E402
from concourse.bass_utils import run_bass_kernel_spmd  # noqa: E402

BF16 = mybir.dt.bfloat16
F32 = mybir.dt.float32
NPBF16 = mybir.dt.np(BF16)
EXP = mybir.ActivationFunctionType.Exp

B, T, C = 4, 2048, 1024
H, DH = 16, 64
HC = 8           # heads per core
CK = C // 128    # 8 contraction chunks over C
TB = T // 128    # 16 key blocks / T row blocks
QC = T // 512    # 4 query chunks
SCALE = 1.0 / np.sqrt(DH)

TRACE = False          # set True (e.g. from test.py) to capture an NTFF profile
LAST_RESULT = None     # BassKernelResults of the last run (exec_time_ns etc.)

_CACHE = None


def _build():
    nc = bacc.Bacc("TRN2", target_bir_lowering=False, debug=False, num_devices=8)

    xT = nc.dram_tensor("xT", [C, T], BF16, kind="ExternalInput")
    wqkv = nc.dram_tensor("wqkv", [C, 3 * 512], BF16, kind="ExternalInput")
    bqk = nc.dram_tensor("bqk", [128, CK], F32, kind="ExternalInput")
    wout = nc.dram_tensor("wout", [512, C], BF16, kind="ExternalInput")
    mskd = nc.dram_tensor("mskd", [128, 128], BF16, kind="ExternalInput")
    seld = nc.dram_tensor("seld", [128, 128], BF16, kind="ExternalInput")
    yT = nc.dram_tensor("yT", [C, T], BF16, kind="ExternalOutput")

    with tile.TileContext(nc) as tc:
        with (
            tc.tile_pool(name="persist", bufs=1) as pp,
            tc.tile_pool(name="sc", bufs=2, space="PSUM") as scp,
            tc.tile_pool(name="oa", bufs=2, space="PSUM") as oap,
            tc.tile_pool(name="pt", bufs=4) as ptp,
            tc.tile_pool(name="nrm", bufs=2) as nrm,
            tc.tile_pool(name="bcp", bufs=2) as bcp,
            tc.tile_pool(name="yst", bufs=3) as yst,
        ):
            # ---- persistent SBUF tiles ----
            XTN = [pp.tile([128, CK, 512], BF16, tag=f"xt{n}", name=f"xt{n}")
                   for n in range(4)]
            WQV = pp.tile([128, CK, 512], BF16, tag="wqv")
            WQM = [pp.tile([128, CK, 128], BF16, tag=f"wqm{m}", name=f"wqm{m}")
                   for m in range(8)]
            QT = [pp.tile([128, T], BF16, tag=f"qt{p}", name=f"qt{p}")
                  for p in range(4)]
            KPP = [pp.tile([128, T], BF16, tag=f"kpp{p}", name=f"kpp{p}")
                   for p in range(4)]
            OT = [pp.tile([128, T], BF16, tag=f"ot{p}", name=f"ot{p}")
                  for p in range(4)]
            # v of a head pair: [v0 (64) | ones (1) | pad (31) | v1 (64)]
            # = 160 cols/pair.  head 0 stationary window = segment[0:128]
            # (out rows 0-63 = out0, row 64 = sumexp0); head 1 window =
            # segment[32:160] (row 32 = sumexp1, rows 64-127 = out1) - the
            # sumexp rows land on 32-aligned partitions for the DVE pulls.
            VA = pp.tile([128, TB, 4 * 160], BF16, tag="va")
            WOUT = pp.tile([128, 4, C], BF16, tag="wout")
            MASKD = pp.tile([128, 128], BF16, tag="mskd")
            SEL = pp.tile([128, 128], BF16, tag="sel")
            RRP = [pp.tile([128, 512], BF16, tag=f"rrp{i}", name=f"rrp{i}")
                   for i in range(2)]
            BQK = pp.tile([128, CK], F32, tag="bqk")

            # ---- input loads ----
            # bulk x / v-weights on the sync queue in first-use order;
            # small cold tensors + qk weights + wout on the scalar queue
            # (ACT is idle until the first exp at ~12us).
            def ld3(src_cols):
                return src_cols.rearrange("(k p) c -> p k c", p=128)

            # first-needed data split small across both queues so the first
            # v-projection can start ~6us in: x cols 0:256 + v-weight dims
            # 0:256 land first, in parallel.
            nc.sync.dma_start(XTN[0][:, :, 0:256], ld3(xT[:, 0:256]))
            nc.scalar.dma_start(WQV[:, :, 0:256], ld3(wqkv[:, 1024:1280]))
            nc.sync.dma_start(XTN[0][:, :, 256:512], ld3(xT[:, 256:512]))
            nc.scalar.dma_start(WQV[:, :, 256:512], ld3(wqkv[:, 1280:1536]))
            nc.scalar.dma_start(WQM[0][:], ld3(wqkv[:, 0:128]))
            nc.scalar.dma_start(WQM[4][:], ld3(wqkv[:, 512:640]))
            nc.scalar.dma_start(BQK[:], bqk[:])
            nc.scalar.dma_start(MASKD[:], mskd[:])
            for n in range(1, 4):
                nc.sync.dma_start(
                    XTN[n][:], ld3(xT[:, n * 512:(n + 1) * 512])
                )
            for m in (1, 5, 2, 6, 3, 7):
                base = (m % 4) * 128 + (512 if m >= 4 else 0)
                nc.scalar.dma_start(WQM[m][:], ld3(wqkv[:, base:base + 128]))
            nc.scalar.dma_start(WOUT[:], wout[:].rearrange("(k p) c -> p k c", p=128))

            # ---- constants ----
            # SEL broadcast-selector: row r of SEL^T @ rrp = rrp[0] for
            # r >= 64 (head 1) and rrp[1] for r < 64 (head 0).
            nc.scalar.dma_start(SEL[:], seld[:])
            nc.vector.memset(RRP[0][:], 0.0)
            nc.vector.memset(RRP[1][:], 0.0)
            # ones + pad columns of VA (cols 64-95 of each pair segment)
            va_seg = VA[:].rearrange("p t (r c) -> p t r c", c=160)
            nc.gpsimd.memset(va_seg[:, :, :, 64:65], 1.0)
            nc.gpsimd.memset(va_seg[:, :, :, 65:96], 0.0)

            # ---- emission helpers ----
            def emit_v(psl, t):
                xtile = XTN[t // 4]
                to = (t % 4) * 128
                for kc in range(CK):
                    nc.tensor.matmul(
                        psl,
                        xtile[:, kc, to:to + 128],
                        WQV[:, kc, :],
                        start=(kc == 0),
                        stop=(kc == CK - 1),
                    )
                src = psl.rearrange("p (r s c) -> p r s c", s=2, c=64)
                dst = VA[:, t, :].rearrange("p (r c) -> p r c", c=160)
                nc.vector.tensor_copy(dst[:, :, 0:64], src[:, :, 0, :])
                nc.vector.tensor_copy(dst[:, :, 96:160], src[:, :, 1, :])

            def emit_vgroup(t2):
                t3 = scp.tile([128, 1024], F32, tag="sc", name="vps")
                emit_v(t3[:, 0:512], t2)
                emit_v(t3[:, 512:1024], t2 + 1)

            def emit_v_half(psl, t, vh):
                # half-width v projection (v dims vh*256..): lets the first
                # matmuls start once only half of x/wv has landed.
                xtile = XTN[t // 4]
                to = (t % 4) * 128
                for kc in range(CK):
                    nc.tensor.matmul(
                        psl,
                        xtile[:, kc, to:to + 128],
                        WQV[:, kc, vh * 256:(vh + 1) * 256],
                        start=(kc == 0),
                        stop=(kc == CK - 1),
                    )
                src = psl.rearrange("p (r s c) -> p r s c", r=2, s=2, c=64)
                dst = VA[:, t, :].rearrange("p (r c) -> p r c", c=160)
                dst = dst[:, 2 * vh:2 * vh + 2, :]
                nc.vector.tensor_copy(dst[:, :, 0:64], src[:, :, 0, :])
                nc.vector.tensor_copy(dst[:, :, 96:160], src[:, :, 1, :])

            def emit_qk(pair, qk, n):
                m = pair + 4 * qk
                psl = scp.tile([128, 1024], F32, tag="sc", name="qkps")[:, 0:512]
                for kc in range(CK):
                    nc.tensor.matmul(
                        psl,
                        WQM[m][:, kc, :],
                        XTN[n][:, kc, :],
                        start=(kc == 0),
                        stop=(kc == CK - 1),
                    )
                ns = slice(n * 512, (n + 1) * 512)
                dst = (QT if qk == 0 else KPP)[pair]
                nc.vector.tensor_scalar_add(dst[:, ns], psl, BQK[:, m:m + 1])

            def emit_yjob(n, mo2):
                t3 = scp.tile([128, 1024], F32, tag="sc", name="yps")
                ns = slice(n * 512, (n + 1) * 512)
                for s in range(2):
                    mo = mo2 + s
                    psl = t3[:, s * 512:(s + 1) * 512]
                    for kc in range(4):
                        nc.tensor.matmul(
                            psl,
                            WOUT[:, kc, mo * 128:(mo + 1) * 128],
                            OT[kc][:, ns],
                            start=(kc == 0),
                            stop=(kc == 3),
                        )
                    ys = yst.tile([128, 512], BF16, tag="ys", name="ys")
                    nc.scalar.copy(ys[:], psl)
                    nc.sync.dma_start(
                        yT[mo * 128:(mo + 1) * 128, ns], ys[:]
                    )

            # ---- normalization ----
            pending_norms = []
            _nrm_ctr = [0]

            def flush_norms():
                while pending_norms:
                    pending_norms.pop(0)()

            def norm_part1(pair, j, oaccj):
                # pull the two sum-of-exp rows out of PSUM, spread both over
                # 64 partitions x 16 with one DMA, reciprocal wide, gather
                # into rows 0 (head 1) / 1 (head 0) of a zero-padded tile.
                rc = nrm.tile([128, 512], F32, tag="rc", name="rc")
                nc.vector.tensor_copy(rc[64:65, :], oaccj[64:65, 0:512])
                nc.vector.tensor_copy(rc[32:33, :], oaccj[32:33, 512:1024])
                rs = nrm.tile([64, 16], F32, tag="rs", name="rs")
                nc.sync.dma_start(out=rs[:, 0:8], in_=rc[64:65, :])
                nc.sync.dma_start(out=rs[:, 8:16], in_=rc[32:33, :])
                nc.vector.reciprocal(rs[:], rs[:])
                rsb = nrm.tile([64, 16], BF16, tag="rsb", name="rsb")
                nc.vector.tensor_copy(rsb[:], rs[:])
                rrp = RRP[_nrm_ctr[0] % 2]
                _nrm_ctr[0] += 1
                nc.sync.dma_start(out=rrp[0:1, 0:512], in_=rsb[:, 0:8])
                nc.sync.dma_start(out=rrp[1:2, 0:512], in_=rsb[:, 8:16])
                return rrp

            def norm_part2(pair, j, oaccj, rrp):
                # replicate 1/sumexp to all partitions with one PE matmul
                # (SEL routes head 0/1 rows), then scale straight into OT.
                js = slice(j * 512, (j + 1) * 512)
                bc = scp.tile([128, 1024], F32, tag="sc", name="bc")
                nc.tensor.matmul(bc[:, 0:512], SEL[:], rrp[:], start=True, stop=True)
                bcv = bcp.tile([128, 512], F32, tag="bcv", name="bcv")
                nc.vector.tensor_copy(bcv[:], bc[:, 0:512])
                nc.vector.tensor_mul(
                    OT[pair][0:64, js], oaccj[0:64, 0:512], bcv[0:64, :]
                )
                nc.vector.tensor_mul(
                    OT[pair][64:128, js], oaccj[64:128, 512:1024], bcv[64:128, :]
                )

            # ---- filler schedule (PE work drained into ACT-bound gaps) ----
            fillers = []

            def drain(k=1):
                for _ in range(min(k, len(fillers))):
                    fillers.pop(0)()

            def F_v(t2):
                return lambda: emit_vgroup(t2)

            def F_qk(pair, qk, n):
                return lambda: emit_qk(pair, qk, n)

            def F_y(n, mo2):
                return lambda: emit_yjob(n, mo2)

            SCHED = {
                (0, 0): [F_qk(0, 0, 1), F_qk(0, 1, 1)],
                (0, 1): [F_v(4), F_qk(0, 0, 2), F_v(6), F_qk(0, 1, 2)],
                (0, 2): [F_v(8), F_v(10), F_qk(0, 0, 3), F_qk(0, 1, 3)],
                (0, 3): [F_v(12), F_v(14), F_qk(1, 0, 0), F_qk(1, 1, 0),
                         F_qk(1, 0, 1), F_qk(1, 1, 1)],
                (1, 0): [F_qk(1, 0, 2), F_qk(1, 1, 2)],
                (1, 1): [F_qk(1, 0, 3), F_qk(1, 1, 3)],
                (1, 2): [F_qk(2, 0, 0), F_qk(2, 1, 0), F_qk(2, 0, 1),
                         F_qk(2, 1, 1)],
                (1, 3): [F_qk(2, 0, 2), F_qk(2, 1, 2), F_qk(2, 0, 3),
                         F_qk(2, 1, 3)],
                (2, 0): [],
                (2, 1): [F_qk(3, 0, 0), F_qk(3, 1, 0)],
                (2, 2): [F_qk(3, 0, 1), F_qk(3, 1, 1), F_qk(3, 0, 2),
                         F_qk(3, 1, 2)],
                (2, 3): [F_qk(3, 0, 3), F_qk(3, 1, 3)],
                (3, 0): [],
                (3, 1): [F_y(0, 0), F_y(0, 2), F_y(0, 4), F_y(0, 6)],
                (3, 2): [F_y(1, 0), F_y(1, 2), F_y(1, 4), F_y(1, 6)],
                (3, 3): [F_y(2, 0), F_y(2, 2), F_y(2, 4), F_y(2, 6)],
            }

            # ---- prologue: enough v/qk for (pair 0, j = 0) ----
            t3 = scp.tile([128, 1024], F32, tag="sc", name="vps")
            emit_v_half(t3[:, 0:256], 0, 0)
            emit_v_half(t3[:, 256:512], 0, 1)
            emit_v_half(t3[:, 512:768], 1, 0)
            emit_v_half(t3[:, 768:1024], 1, 1)
            emit_qk(0, 0, 0)
            emit_vgroup(2)
            emit_qk(0, 1, 0)

            # ---- attention ----
            for pair in range(4):
                for j in range(QC):
                    fillers.extend(SCHED[(pair, j)])
                    nb = 4 * (j + 1)
                    js0 = j * 512
                    oaccj = oap.tile([128, 1024], F32, tag="oa", name="oaccj")
                    for k in range(0, nb, 2):
                        scs = []
                        pts = []
                        for i in (k, k + 1):
                            d = i - 4 * j
                            c0 = 128 * max(d, 0)
                            sc = scp.tile([128, 1024], F32, tag="sc", name="sc")
                            nc.tensor.matmul(
                                sc[:, c0:512],
                                KPP[pair][0:64, i * 128:(i + 1) * 128],
                                QT[pair][0:64, js0 + c0:js0 + 512],
                                start=True, stop=True,
                            )
                            nc.tensor.matmul(
                                sc[:, 512 + c0:1024],
                                KPP[pair][64:128, i * 128:(i + 1) * 128],
                                QT[pair][64:128, js0 + c0:js0 + 512],
                                start=True, stop=True,
                            )
                            pt = ptp.tile([128, 1024], BF16, tag="pt")
                            scv = sc.rearrange("p (s c) -> p s c", c=512)
                            ptv = pt.rearrange("p (s c) -> p s c", c=512)
                            if d >= 0:
                                # additive causal mask (-1e4 above diagonal)
                                # applied to the scores in PSUM, so PV needs
                                # only the exp result.
                                for s in range(2):
                                    sl = scv[:, s, c0:c0 + 128]
                                    nc.vector.tensor_add(sl, sl, MASKD[:])
                            nc.scalar.activation(
                                ptv[:, :, c0:512], scv[:, :, c0:512], EXP
                            )
                            scs.append(sc)
                            pts.append(pt)
                        if k == 2:
                            flush_norms()
                        # pair 3's fillers are out-proj jobs that read OT
                        # slices written by the k==2 norm flush - draining
                        # them at k==0 would bind to stale OT.
                        if pair != 3 or k >= 2:
                            drain(1)
                        for idx, i in enumerate((k, k + 1)):
                            d = i - 4 * j
                            c0 = 128 * max(d, 0)
                            pt = pts[idx]
                            nc.tensor.matmul(
                                oaccj[:, c0:512],
                                VA[:, i, pair * 160:pair * 160 + 128],
                                pt[:, c0:512],
                                start=(i == 0), stop=(i == nb - 1),
                            )
                            nc.tensor.matmul(
                                oaccj[:, 512 + c0:1024],
                                VA[:, i, pair * 160 + 32:pair * 160 + 160],
                                pt[:, 512 + c0:1024],
                                start=(i == 0), stop=(i == nb - 1),
                            )
                    rrp = norm_part1(pair, j, oaccj)
                    pending_norms.append(
                        lambda pair=pair, j=j, oaccj=oaccj, rrp=rrp:
                        norm_part2(pair, j, oaccj, rrp)
                    )

            flush_norms()
            for mo2 in range(0, 8, 2):
                emit_yjob(3, mo2)

    nc.compile()
    return nc


def _make_maskd():
    p = np.arange(128)[:, None]
    f = np.arange(128)[None, :]
    return np.where(p <= f, 0.0, -10000.0).astype(np.float32).astype(NPBF16)


def _make_seld():
    s = np.zeros((128, 128), np.float32)
    s[0, 0:64] = 1.0     # rrp row 0 = 1/sumexp0 -> bc rows 0-63 (head 0)
    s[1, 64:128] = 1.0   # rrp row 1 = 1/sumexp1 -> bc rows 64-127 (head 1)
    return s.astype(NPBF16)


def kernel(x, w_qkv, b_qkv, w_out, b_out):
    global _CACHE, LAST_RESULT
    x = np.asarray(x, np.float32)
    w_qkv = np.asarray(w_qkv, np.float32)
    b_qkv = np.asarray(b_qkv, np.float32)
    w_out = np.asarray(w_out, np.float32)
    b_out = np.asarray(b_out, np.float32)

    if _CACHE is None:
        _CACHE = _build()
    nc = _CACHE

    maskd = _make_maskd()
    seld = _make_seld()
    in_maps = []
    bias_corr = []
    for core in range(8):
        b = core // 2
        g = core % 2
        sl = slice(g * 512, (g + 1) * 512)
        wq = w_qkv[:, 0:1024][:, sl] * SCALE
        wk = w_qkv[:, 1024:2048][:, sl]
        wv = w_qkv[:, 2048:3072][:, sl]
        wqkv_c = np.ascontiguousarray(
            np.concatenate([wq, wk, wv], axis=1).astype(NPBF16)
        )
        bq = b_qkv[0:1024][sl] * SCALE
        bk = b_qkv[1024:2048][sl]
        bqk_c = np.ascontiguousarray(
            np.concatenate([bq, bk]).reshape(CK, 128).T.astype(np.float32)
        )
        # v-bias folded through the out-projection on the host
        bias_corr.append(w_out[sl, :].T @ b_qkv[2048:3072][sl])
        in_maps.append(
            {
                "xT": np.ascontiguousarray(x[b].T.astype(NPBF16)),
                "wqkv": wqkv_c,
                "bqk": bqk_c,
                "wout": np.ascontiguousarray(w_out[sl, :].astype(NPBF16)),
                "mskd": maskd,
                "seld": seld,
            }
        )

    res = run_bass_kernel_spmd(nc, in_maps, core_ids=list(range(8)), trace=TRACE)
    LAST_RESULT = res

    out = np.empty((B, T, C), np.float32)
    for b in range(B):
        acc = res.results[2 * b]["yT"].astype(np.float32) + res.results[
            2 * b + 1
        ]["yT"].astype(np.float32)
        bias = b_out + bias_corr[2 * b] + bias_corr[2 * b + 1]
        out[b] = acc.T + bias[None, :]
    return out


# revision 26
# speedup vs baseline: 1.0545x; 1.0545x over previous
"""Causal self-attention (B=4, T=2048, C=1024, H=16) on 8 TRN2 NeuronCores.

Sharding: core = (batch, head-group) on a 4x2 grid. Each core computes the
attention output of 8 heads for one batch element plus its partial out-proj
(y^T = w_out_slice^T @ out_heads^T); the two head-groups of a batch are summed
on the host (the "out_proj all-reduce"), where the final bias is also added.

On-chip dataflow is fully transposed so no transposes are ever needed:
  qk^T  = w_qkv_slice^T @ x^T          (C on partitions)
  v     = x @ w_v_slice                (T on partitions, natural)
  S^T   = k_h @ q_h^T                  (k-positions on partitions; the two
                                        heads of a pair run CONCURRENTLY as
                                        64-row PE tiles - q/k of head 0 live
                                        in partitions 0-63, head 1 in 64-127)
  P^T   = exp(S^T) * causal_mask       (no max-subtraction: scores ~ N(0,1);
                                        mask multiplies run on GpSimd)
  outT  = [v|1]^T @ P^T                (ones column accumulates sum-of-exp;
                                        the v block of a pair is laid out
                                        [v0|1|v1] so head 0 uses window
                                        [0:128] (out rows 0-63 + se at 64)
                                        and head 1 uses window [1:129]
                                        (se at 63... row 64-127 = out1) -
                                        both heads' outputs land on their own
                                        partitions, so the normalized result
                                        is written straight into OT by DVE)
  y^T   = w_out_slice^T @ (outT/sumexp)

Causal trimming: for a diagonal key block at offset d (0-3) only query
columns >= 128*d can attend it, so score matmuls / exp / mask / PV all
operate on the trailing 512-128*d columns only.

The PE queue is statically interleaved: projection matmul groups (v, qk of
the next pair, out-proj once a query chunk is fully normalized) are drained
one per score/PV iteration so the PE never idles while ACT works through the
exp stream, and the HAM clock gate never re-throttles.
"""

import sys
import types

if "/opt/trn_rl_repo" not in sys.path:
    sys.path.insert(0, "/opt/trn_rl_repo")

import numpy as np


def _install_ntff_hook_shim():
    """antenv.axon_hooks is missing in this image; provide it so that
    run_bass_kernel_spmd(trace=True) can capture NTFF profiles."""
    if "antenv.axon_hooks" in sys.modules:
        return
    try:
        from trn_agent_boot.trn_boot import _ntff_profile_via_ctypes

        hook = _ntff_profile_via_ctypes("/opt/axon/libaxon_pjrt.so")
    except Exception:
        hook = None
    m = types.ModuleType("antenv.axon_hooks")
    m.get_axon_ntff_profile_hook = lambda: hook
    sys.modules["antenv.axon_hooks"] = m


_install_ntff_hook_shim()

import concourse.bass as bass  # noqa: E402
from concourse import bacc  # noqa: E402
import concourse.mybir as mybir  # noqa: E402
import concourse.tile as tile  # noqa: E402
from concourse.bass_utils import run_bass_kernel_spmd  # noqa: E402

BF16 = mybir.dt.bfloat16
F32 = mybir.dt.float32
NPBF16 = mybir.dt.np(BF16)
EXP = mybir.ActivationFunctionType.Exp

B, T, C = 4, 2048, 1024
H, DH = 16, 64
HC = 8           # heads per core
CK = C // 128    # 8 contraction chunks over C
TB = T // 128    # 16 key blocks / T row blocks
QC = T // 512    # 4 query chunks
SCALE = 1.0 / np.sqrt(DH)

TRACE = False          # set True (e.g. from test.py) to capture an NTFF profile
LAST_RESULT = None     # BassKernelResults of the last run (exec_time_ns etc.)

_CACHE = None


def _build():
    nc = bacc.Bacc("TRN2", target_bir_lowering=False, debug=False, num_devices=8)

    xT = nc.dram_tensor("xT", [C, T], BF16, kind="ExternalInput")
    wqkv = nc.dram_tensor("wqkv", [C, 3 * 512], BF16, kind="ExternalInput")
    bqk = nc.dram_tensor("bqk", [128, CK], F32, kind="ExternalInput")
    wout = nc.dram_tensor("wout", [512, C], BF16, kind="ExternalInput")
    mskd = nc.dram_tensor("mskd", [128, 128], BF16, kind="ExternalInput")
    seld = nc.dram_tensor("seld", [128, 128], BF16, kind="ExternalInput")
    yT = nc.dram_tensor("yT", [C, T], BF16, kind="ExternalOutput")

    with tile.TileContext(nc) as tc:
        with (
            tc.tile_pool(name="persist", bufs=1) as pp,
            tc.tile_pool(name="sc", bufs=2, space="PSUM") as scp,
            tc.tile_pool(name="oa", bufs=2, space="PSUM") as oap,
            tc.tile_pool(name="pt", bufs=4) as ptp,
            tc.tile_pool(name="nrm", bufs=2) as nrm,
            tc.tile_pool(name="bcp", bufs=2) as bcp,
            tc.tile_pool(name="yst", bufs=3) as yst,
        ):
            # ---- persistent SBUF tiles ----
            XTN = [pp.tile([128, CK, 512], BF16, tag=f"xt{n}", name=f"xt{n}")
                   for n in range(4)]
            WQV = pp.tile([128, CK, 512], BF16, tag="wqv")
            WQM = [pp.tile([128, CK, 128], BF16, tag=f"wqm{m}", name=f"wqm{m}")
                   for m in range(8)]
            QT = [pp.tile([128, T], BF16, tag=f"qt{p}", name=f"qt{p}")
                  for p in range(4)]
            KPP = [pp.tile([128, T], BF16, tag=f"kpp{p}", name=f"kpp{p}")
                   for p in range(4)]
            OT = [pp.tile([128, T], BF16, tag=f"ot{p}", name=f"ot{p}")
                  for p in range(4)]
            # v of a head pair: [v0 (64) | ones (1) | pad (31) | v1 (64)]
            # = 160 cols/pair.  head 0 stationary window = segment[0:128]
            # (out rows 0-63 = out0, row 64 = sumexp0); head 1 window =
            # segment[32:160] (row 32 = sumexp1, rows 64-127 = out1) - the
            # sumexp rows land on 32-aligned partitions for the DVE pulls.
            VA = pp.tile([128, TB, 4 * 160], BF16, tag="va")
            WOUT = pp.tile([128, 4, C], BF16, tag="wout")
            MASKD = pp.tile([128, 128], BF16, tag="mskd")
            SEL = pp.tile([128, 128], BF16, tag="sel")
            RRP = [pp.tile([128, 512], BF16, tag=f"rrp{i}", name=f"rrp{i}")
                   for i in range(2)]
            BQK = pp.tile([128, CK], F32, tag="bqk")

            # ---- input loads ----
            # bulk x / v-weights on the sync queue in first-use order;
            # small cold tensors + qk weights + wout on the scalar queue
            # (ACT is idle until the first exp at ~12us).
            def ld3(src_cols):
                return src_cols.rearrange("(k p) c -> p k c", p=128)

            # first-needed data split small across both queues so the first
            # v-projection can start ~6us in: x cols 0:256 + v-weight dims
            # 0:256 land first, in parallel.
            nc.sync.dma_start(XTN[0][:, :, 0:256], ld3(xT[:, 0:256]))
            nc.scalar.dma_start(WQV[:, :, 0:256], ld3(wqkv[:, 1024:1280]))
            nc.sync.dma_start(XTN[0][:, :, 256:512], ld3(xT[:, 256:512]))
            nc.scalar.dma_start(WQV[:, :, 256:512], ld3(wqkv[:, 1280:1536]))
            nc.scalar.dma_start(WQM[0][:], ld3(wqkv[:, 0:128]))
            nc.scalar.dma_start(WQM[4][:], ld3(wqkv[:, 512:640]))
            nc.scalar.dma_start(BQK[:], bqk[:])
            nc.scalar.dma_start(MASKD[:], mskd[:])
            for n in range(1, 4):
                nc.sync.dma_start(
                    XTN[n][:], ld3(xT[:, n * 512:(n + 1) * 512])
                )
            for m in (1, 5, 2, 6, 3, 7):
                base = (m % 4) * 128 + (512 if m >= 4 else 0)
                nc.scalar.dma_start(WQM[m][:], ld3(wqkv[:, base:base + 128]))
            nc.scalar.dma_start(WOUT[:], wout[:].rearrange("(k p) c -> p k c", p=128))

            # ---- constants ----
            # SEL broadcast-selector: row r of SEL^T @ rrp = rrp[0] for
            # r >= 64 (head 1) and rrp[1] for r < 64 (head 0).
            nc.scalar.dma_start(SEL[:], seld[:])
            nc.vector.memset(RRP[0][:], 0.0)
            nc.vector.memset(RRP[1][:], 0.0)
            # ones + pad columns of VA (cols 64-95 of each pair segment)
            va_seg = VA[:].rearrange("p t (r c) -> p t r c", c=160)
            nc.gpsimd.memset(va_seg[:, :, :, 64:65], 1.0)
            nc.gpsimd.memset(va_seg[:, :, :, 65:96], 0.0)

            # ---- emission helpers ----
            def emit_v(psl, t):
                xtile = XTN[t // 4]
                to = (t % 4) * 128
                for kc in range(CK):
                    nc.tensor.matmul(
                        psl,
                        xtile[:, kc, to:to + 128],
                        WQV[:, kc, :],
                        start=(kc == 0),
                        stop=(kc == CK - 1),
                    )
                src = psl.rearrange("p (r s c) -> p r s c", s=2, c=64)
                dst = VA[:, t, :].rearrange("p (r c) -> p r c", c=160)
                nc.vector.tensor_copy(dst[:, :, 0:64], src[:, :, 0, :])
                nc.vector.tensor_copy(dst[:, :, 96:160], src[:, :, 1, :])

            def emit_vgroup(t2):
                t3 = scp.tile([128, 1024], F32, tag="sc", name="vps")
                emit_v(t3[:, 0:512], t2)
                emit_v(t3[:, 512:1024], t2 + 1)

            def emit_v_half(psl, t, vh):
                # half-width v projection (v dims vh*256..): lets the first
                # matmuls start once only half of x/wv has landed.
                xtile = XTN[t // 4]
                to = (t % 4) * 128
                for kc in range(CK):
                    nc.tensor.matmul(
                        psl,
                        xtile[:, kc, to:to + 128],
                        WQV[:, kc, vh * 256:(vh + 1) * 256],
                        start=(kc == 0),
                        stop=(kc == CK - 1),
                    )
                src = psl.rearrange("p (r s c) -> p r s c", r=2, s=2, c=64)
                dst = VA[:, t, :].rearrange("p (r c) -> p r c", c=160)
                dst = dst[:, 2 * vh:2 * vh + 2, :]
                nc.vector.tensor_copy(dst[:, :, 0:64], src[:, :, 0, :])
                nc.vector.tensor_copy(dst[:, :, 96:160], src[:, :, 1, :])

            def emit_qk(pair, qk, n):
                m = pair + 4 * qk
                psl = scp.tile([128, 1024], F32, tag="sc", name="qkps")[:, 0:512]
                for kc in range(CK):
                    nc.tensor.matmul(
                        psl,
                        WQM[m][:, kc, :],
                        XTN[n][:, kc, :],
                        start=(kc == 0),
                        stop=(kc == CK - 1),
                    )
                ns = slice(n * 512, (n + 1) * 512)
                dst = (QT if qk == 0 else KPP)[pair]
                nc.vector.tensor_scalar_add(dst[:, ns], psl, BQK[:, m:m + 1])

            def emit_yjob(n, mo2):
                t3 = scp.tile([128, 1024], F32, tag="sc", name="yps")
                ns = slice(n * 512, (n + 1) * 512)
                for s in range(2):
                    mo = mo2 + s
                    psl = t3[:, s * 512:(s + 1) * 512]
                    for kc in range(4):
                        nc.tensor.matmul(
                            psl,
                            WOUT[:, kc, mo * 128:(mo + 1) * 128],
                            OT[kc][:, ns],
                            start=(kc == 0),
                            stop=(kc == 3),
                        )
                    ys = yst.tile([128, 512], BF16, tag="ys", name="ys")
                    nc.scalar.copy(ys[:], psl)
                    nc.sync.dma_start(
                        yT[mo * 128:(mo + 1) * 128, ns], ys[:]
                    )

            # ---- normalization ----
            pending_norms = []
            _nrm_ctr = [0]

            def flush_norms():
                while pending_norms:
                    pending_norms.pop(0)()

            def norm_part1(pair, j, oaccj):
                # pull the two sum-of-exp rows out of PSUM, spread both over
                # 64 partitions x 16 with one DMA, reciprocal wide, gather
                # into rows 0 (head 1) / 1 (head 0) of a zero-padded tile.
                rc = nrm.tile([128, 512], F32, tag="rc", name="rc")
                nc.vector.tensor_copy(rc[64:65, :], oaccj[64:65, 0:512])
                nc.vector.tensor_copy(rc[32:33, :], oaccj[32:33, 512:1024])
                rs = nrm.tile([64, 16], F32, tag="rs", name="rs")
                nc.sync.dma_start(out=rs[:, 0:8], in_=rc[64:65, :])
                nc.sync.dma_start(out=rs[:, 8:16], in_=rc[32:33, :])
                nc.vector.reciprocal(rs[:], rs[:])
                rsb = nrm.tile([64, 16], BF16, tag="rsb", name="rsb")
                nc.vector.tensor_copy(rsb[:], rs[:])
                rrp = RRP[_nrm_ctr[0] % 2]
                _nrm_ctr[0] += 1
                nc.sync.dma_start(out=rrp[0:1, 0:512], in_=rsb[:, 0:8])
                nc.sync.dma_start(out=rrp[1:2, 0:512], in_=rsb[:, 8:16])
                return rrp

            def norm_part2(pair, j, oaccj, rrp):
                # replicate 1/sumexp to all partitions with one PE matmul
                # (SEL routes head 0/1 rows), then scale straight into OT.
                js = slice(j * 512, (j + 1) * 512)
                bc = scp.tile([128, 1024], F32, tag="sc", name="bc")
                nc.tensor.matmul(bc[:, 0:512], SEL[:], rrp[:], start=True, stop=True)
                bcv = bcp.tile([128, 512], F32, tag="bcv", name="bcv")
                nc.vector.tensor_copy(bcv[:], bc[:, 0:512])
                nc.vector.tensor_mul(
                    OT[pair][0:64, js], oaccj[0:64, 0:512], bcv[0:64, :]
                )
                nc.vector.tensor_mul(
                    OT[pair][64:128, js], oaccj[64:128, 512:1024], bcv[64:128, :]
                )

            # ---- filler schedule (PE work drained into ACT-bound gaps) ----
            fillers = []

            def drain(k=1):
                for _ in range(min(k, len(fillers))):
                    fillers.pop(0)()

            def F_v(t2):
                return lambda: emit_vgroup(t2)

            def F_qk(pair, qk, n):
                return lambda: emit_qk(pair, qk, n)

            def F_y(n, mo2):
                return lambda: emit_yjob(n, mo2)

            SCHED = {
                (0, 0): [F_qk(0, 0, 1), F_qk(0, 1, 1)],
                (0, 1): [F_v(4), F_qk(0, 0, 2), F_v(6), F_qk(0, 1, 2)],
                (0, 2): [F_v(8), F_v(10), F_qk(0, 0, 3), F_qk(0, 1, 3)],
                (0, 3): [F_v(12), F_v(14), F_qk(1, 0, 0), F_qk(1, 1, 0),
                         F_qk(1, 0, 1), F_qk(1, 1, 1)],
                (1, 0): [F_qk(1, 0, 2), F_qk(1, 1, 2)],
                (1, 1): [F_qk(1, 0, 3), F_qk(1, 1, 3)],
                (1, 2): [F_qk(2, 0, 0), F_qk(2, 1, 0), F_qk(2, 0, 1),
                         F_qk(2, 1, 1)],
                (1, 3): [F_qk(2, 0, 2), F_qk(2, 1, 2), F_qk(2, 0, 3),
                         F_qk(2, 1, 3)],
                (2, 0): [],
                (2, 1): [F_qk(3, 0, 0), F_qk(3, 1, 0)],
                (2, 2): [F_qk(3, 0, 1), F_qk(3, 1, 1), F_qk(3, 0, 2),
                         F_qk(3, 1, 2)],
                (2, 3): [F_qk(3, 0, 3), F_qk(3, 1, 3)],
                (3, 0): [],
                (3, 1): [F_y(0, 0), F_y(0, 2), F_y(0, 4), F_y(0, 6)],
                (3, 2): [F_y(1, 0), F_y(1, 2), F_y(1, 4), F_y(1, 6)],
                (3, 3): [F_y(2, 0), F_y(2, 2), F_y(2, 4), F_y(2, 6)],
            }

            # ---- prologue: enough v/qk for (pair 0, j = 0) ----
            t3 = scp.tile([128, 1024], F32, tag="sc", name="vps")
            emit_v_half(t3[:, 0:256], 0, 0)
            emit_v_half(t3[:, 256:512], 0, 1)
            emit_v_half(t3[:, 512:768], 1, 0)
            emit_v_half(t3[:, 768:1024], 1, 1)
            emit_qk(0, 0, 0)
            emit_vgroup(2)
            emit_qk(0, 1, 0)

            # ---- attention ----
            for pair in range(4):
                for j in range(QC):
                    fillers.extend(SCHED[(pair, j)])
                    nb = 4 * (j + 1)
                    js0 = j * 512
                    oaccj = oap.tile([128, 1024], F32, tag="oa", name="oaccj")
                    for k in range(0, nb, 2):
                        scs = []
                        pts = []
                        for i in (k, k + 1):
                            d = i - 4 * j
                            c0 = 128 * max(d, 0)
                            sc = scp.tile([128, 1024], F32, tag="sc", name="sc")
                            nc.tensor.matmul(
                                sc[:, c0:512],
                                KPP[pair][0:64, i * 128:(i + 1) * 128],
                                QT[pair][0:64, js0 + c0:js0 + 512],
                                start=True, stop=True,
                            )
                            nc.tensor.matmul(
                                sc[:, 512 + c0:1024],
                                KPP[pair][64:128, i * 128:(i + 1) * 128],
                                QT[pair][64:128, js0 + c0:js0 + 512],
                                start=True, stop=True,
                            )
                            pt = ptp.tile([128, 1024], BF16, tag="pt")
                            scv = sc.rearrange("p (s c) -> p s c", c=512)
                            ptv = pt.rearrange("p (s c) -> p s c", c=512)
                            nc.scalar.activation(
                                ptv[:, :, c0:512], scv[:, :, c0:512], EXP
                            )
                            if d >= 0:
                                # multiplicative causal mask on the idle
                                # GpSimd engine, off the DVE/ACT queues
                                for s in range(2):
                                    sl = ptv[:, s, c0:c0 + 128]
                                    nc.gpsimd.tensor_mul(sl, sl, MASKD[:])
                            scs.append(sc)
                            pts.append(pt)
                        if k == 2:
                            flush_norms()
                        # pair 3's fillers are out-proj jobs that read OT
                        # slices written by the k==2 norm flush - draining
                        # them at k==0 would bind to stale OT.
                        if pair != 3 or k >= 2:
                            drain(1)
                        for idx, i in enumerate((k, k + 1)):
                            d = i - 4 * j
                            c0 = 128 * max(d, 0)
                            pt = pts[idx]
                            nc.tensor.matmul(
                                oaccj[:, c0:512],
                                VA[:, i, pair * 160:pair * 160 + 128],
                                pt[:, c0:512],
                                start=(i == 0), stop=(i == nb - 1),
                            )
                            nc.tensor.matmul(
                                oaccj[:, 512 + c0:1024],
                                VA[:, i, pair * 160 + 32:pair * 160 + 160],
                                pt[:, 512 + c0:1024],
                                start=(i == 0), stop=(i == nb - 1),
                            )
                    rrp = norm_part1(pair, j, oaccj)
                    pending_norms.append(
                        lambda pair=pair, j=j, oaccj=oaccj, rrp=rrp:
                        norm_part2(pair, j, oaccj, rrp)
                    )

            flush_norms()
            for mo2 in range(0, 8, 2):
                emit_yjob(3, mo2)

    nc.compile()
    return nc


def _make_maskd():
    p = np.arange(128)[:, None]
    f = np.arange(128)[None, :]
    return (p <= f).astype(np.float32).astype(NPBF16)


def _make_seld():
    s = np.zeros((128, 128), np.float32)
    s[0, 0:64] = 1.0     # rrp row 0 = 1/sumexp0 -> bc rows 0-63 (head 0)
    s[1, 64:128] = 1.0   # rrp row 1 = 1/sumexp1 -> bc rows 64-127 (head 1)
    return s.astype(NPBF16)


def kernel(x, w_qkv, b_qkv, w_out, b_out):
    global _CACHE, LAST_RESULT
    x = np.asarray(x, np.float32)
    w_qkv = np.asarray(w_qkv, np.float32)
    b_qkv = np.asarray(b_qkv, np.float32)
    w_out = np.asarray(w_out, np.float32)
    b_out = np.asarray(b_out, np.float32)

    if _CACHE is None:
        _CACHE = _build()
    nc = _CACHE

    maskd = _make_maskd()
    seld = _make_seld()
    in_maps = []
    bias_corr = []
    for core in range(8):
        b = core // 2
        g = core % 2
        sl = slice(g * 512, (g + 1) * 512)
        wq = w_qkv[:, 0:1024][:, sl] * SCALE
        wk = w_qkv[:, 1024:2048][:, sl]
        wv = w_qkv[:, 2048:3072][:, sl]
        wqkv_c = np.ascontiguousarray(
            np.concatenate([wq, wk, wv], axis=1).astype(NPBF16)
        )
        bq = b_qkv[0:1024][sl] * SCALE
        bk = b_qkv[1024:2048][sl]
        bqk_c = np.ascontiguousarray(
            np.concatenate([bq, bk]).reshape(CK, 128).T.astype(np.float32)
        )
        # v-bias folded through the out-projection on the host
        bias_corr.append(w_out[sl, :].T @ b_qkv[2048:3072][sl])
        in_maps.append(
            {
                "xT": np.ascontiguousarray(x[b].T.astype(NPBF16)),
                "wqkv": wqkv_c,
                "bqk": bqk_c,
                "wout": np.ascontiguousarray(w_out[sl, :].astype(NPBF16)),
                "mskd": maskd,
                "seld": seld,
            }
        )

    res = run_bass_kernel_spmd(nc, in_maps, core_ids=list(range(8)), trace=TRACE)
    LAST_RESULT = res

    out = np.empty((B, T, C), np.float32)
    for b in range(B):
        acc = res.results[2 * b]["yT"].astype(np.float32) + res.results[
            2 * b + 1
        ]["yT"].astype(np.float32)
        bias = b_out + bias_corr[2 * b] + bias_corr[2 * b + 1]
        out[b] = acc.T + bias[None, :]
    return out


# revision 30
# speedup vs baseline: 1.1997x; 1.1377x over previous
"""Causal self-attention (B=4, T=2048, C=1024, H=16) on 8 TRN2 NeuronCores.

Sharding: core = (batch, head-group) on a 4x2 grid. Each core computes the
attention output of 8 heads for one batch element plus its partial out-proj
(y^T = w_out_slice^T @ out_heads^T); the two head-groups of a batch are summed
on the host (the "out_proj all-reduce"), where the final bias is also added.

On-chip dataflow is fully transposed so no transposes are ever needed:
  qk^T  = w_qkv_slice^T @ x^T          (C on partitions)
  v     = x @ w_v_slice                (T on partitions, natural)
  S^T   = k_h @ q_h^T                  (k-positions on partitions; the two
                                        heads of a pair run CONCURRENTLY as
                                        64-row PE tiles - q/k of head 0 live
                                        in partitions 0-63, head 1 in 64-127)
  P^T   = exp(S^T) * causal_mask       (no max-subtraction: scores ~ N(0,1);
                                        mask multiplies run on GpSimd)
  outT  = [v|1]^T @ P^T                (ones column accumulates sum-of-exp;
                                        the v block of a pair is laid out
                                        [v0|1|v1] so head 0 uses window
                                        [0:128] (out rows 0-63 + se at 64)
                                        and head 1 uses window [1:129]
                                        (se at 63... row 64-127 = out1) -
                                        both heads' outputs land on their own
                                        partitions, so the normalized result
                                        is written straight into OT by DVE)
  y^T   = w_out_slice^T @ (outT/sumexp)

Causal trimming: for a diagonal key block at offset d (0-3) only query
columns >= 128*d can attend it, so score matmuls / exp / mask / PV all
operate on the trailing 512-128*d columns only.

The PE queue is statically interleaved: projection matmul groups (v, qk of
the next pair, out-proj once a query chunk is fully normalized) are drained
one per score/PV iteration so the PE never idles while ACT works through the
exp stream, and the HAM clock gate never re-throttles.
"""

import sys
import types

if "/opt/trn_rl_repo" not in sys.path:
    sys.path.insert(0, "/opt/trn_rl_repo")

import numpy as np


def _install_ntff_hook_shim():
    """antenv.axon_hooks is missing in this image; provide it so that
    run_bass_kernel_spmd(trace=True) can capture NTFF profiles."""
    if "antenv.axon_hooks" in sys.modules:
        return
    try:
        from trn_agent_boot.trn_boot import _ntff_profile_via_ctypes

        hook = _ntff_profile_via_ctypes("/opt/axon/libaxon_pjrt.so")
    except Exception:
        hook = None
    m = types.ModuleType("antenv.axon_hooks")
    m.get_axon_ntff_profile_hook = lambda: hook
    sys.modules["antenv.axon_hooks"] = m


_install_ntff_hook_shim()

import concourse.bass as bass  # noqa: E402
from concourse import bacc  # noqa: E402
import concourse.mybir as mybir  # noqa: E402
import concourse.tile as tile  # noqa: E402
from concourse.bass_utils import run_bass_kernel_spmd  # noqa: E402

BF16 = mybir.dt.bfloat16
F32 = mybir.dt.float32
NPBF16 = mybir.dt.np(BF16)
EXP = mybir.ActivationFunctionType.Exp

B, T, C = 4, 2048, 1024
H, DH = 16, 64
HC = 8           # heads per core
CK = C // 128    # 8 contraction chunks over C
TB = T // 128    # 16 key blocks / T row blocks
QC = T // 512    # 4 query chunks
SCALE = 1.0 / np.sqrt(DH)

TRACE = False          # set True (e.g. from test.py) to capture an NTFF profile
LAST_RESULT = None     # BassKernelResults of the last run (exec_time_ns etc.)

_CACHE = None


def _build():
    nc = bacc.Bacc("TRN2", target_bir_lowering=False, debug=False, num_devices=8)

    xT = nc.dram_tensor("xT", [C, T], BF16, kind="ExternalInput")
    wqkv = nc.dram_tensor("wqkv", [C, 3 * 512], BF16, kind="ExternalInput")
    bqk = nc.dram_tensor("bqk", [128, CK], F32, kind="ExternalInput")
    wout = nc.dram_tensor("wout", [512, C], BF16, kind="ExternalInput")
    mskd = nc.dram_tensor("mskd", [128, 128], BF16, kind="ExternalInput")
    seld = nc.dram_tensor("seld", [128, 128], BF16, kind="ExternalInput")
    yT = nc.dram_tensor("yT", [C, T], BF16, kind="ExternalOutput")

    with tile.TileContext(nc) as tc:
        with (
            tc.tile_pool(name="persist", bufs=1) as pp,
            tc.tile_pool(name="sc", bufs=3, space="PSUM") as scp,
            tc.tile_pool(name="oa", bufs=1, space="PSUM") as oap,
            tc.tile_pool(name="pt", bufs=4) as ptp,
            tc.tile_pool(name="nrm", bufs=2) as nrm,
            tc.tile_pool(name="bcp", bufs=2) as bcp,
            tc.tile_pool(name="yst", bufs=3) as yst,
        ):
            # ---- persistent SBUF tiles ----
            XTN = [pp.tile([128, CK, 512], BF16, tag=f"xt{n}", name=f"xt{n}")
                   for n in range(4)]
            WQV = pp.tile([128, CK, 512], BF16, tag="wqv")
            WQM = [pp.tile([128, CK, 128], BF16, tag=f"wqm{m}", name=f"wqm{m}")
                   for m in range(8)]
            QT = [pp.tile([128, T], BF16, tag=f"qt{p}", name=f"qt{p}")
                  for p in range(4)]
            KPP = [pp.tile([128, T], BF16, tag=f"kpp{p}", name=f"kpp{p}")
                   for p in range(4)]
            OT = [pp.tile([128, T], BF16, tag=f"ot{p}", name=f"ot{p}")
                  for p in range(4)]
            # v of a head pair: [v0 (64) | ones (1) | pad (31) | v1 (64)]
            # = 160 cols/pair.  head 0 stationary window = segment[0:128]
            # (out rows 0-63 = out0, row 64 = sumexp0); head 1 window =
            # segment[32:160] (row 32 = sumexp1, rows 64-127 = out1) - the
            # sumexp rows land on 32-aligned partitions for the DVE pulls.
            VA = pp.tile([128, TB, 4 * 160], BF16, tag="va")
            WOUT = pp.tile([128, 4, C], BF16, tag="wout")
            MASKD = pp.tile([128, 128], BF16, tag="mskd")
            SEL = pp.tile([128, 128], BF16, tag="sel")
            RRP = [pp.tile([128, 512], BF16, tag=f"rrp{i}", name=f"rrp{i}")
                   for i in range(2)]
            BQK = pp.tile([128, CK], F32, tag="bqk")

            # ---- input loads ----
            # bulk x / v-weights on the sync queue in first-use order;
            # small cold tensors + qk weights + wout on the scalar queue
            # (ACT is idle until the first exp at ~12us).
            def ld3(src_cols):
                return src_cols.rearrange("(k p) c -> p k c", p=128)

            # first-needed data split small across both queues so the first
            # v-projection can start ~6us in: x cols 0:256 + v-weight dims
            # 0:256 land first, in parallel.
            nc.sync.dma_start(XTN[0][:, :, 0:256], ld3(xT[:, 0:256]))
            nc.scalar.dma_start(WQV[:, :, 0:256], ld3(wqkv[:, 1024:1280]))
            nc.sync.dma_start(XTN[0][:, :, 256:512], ld3(xT[:, 256:512]))
            nc.scalar.dma_start(WQV[:, :, 256:512], ld3(wqkv[:, 1280:1536]))
            nc.scalar.dma_start(WQM[0][:], ld3(wqkv[:, 0:128]))
            nc.scalar.dma_start(WQM[4][:], ld3(wqkv[:, 512:640]))
            nc.scalar.dma_start(BQK[:], bqk[:])
            nc.scalar.dma_start(MASKD[:], mskd[:])
            for n in range(1, 4):
                nc.sync.dma_start(
                    XTN[n][:], ld3(xT[:, n * 512:(n + 1) * 512])
                )
            for m in (1, 5, 2, 6, 3, 7):
                base = (m % 4) * 128 + (512 if m >= 4 else 0)
                nc.scalar.dma_start(WQM[m][:], ld3(wqkv[:, base:base + 128]))
            nc.scalar.dma_start(WOUT[:], wout[:].rearrange("(k p) c -> p k c", p=128))

            # ---- constants ----
            # SEL broadcast-selector: row r of SEL^T @ rrp = rrp[0] for
            # r >= 64 (head 1) and rrp[1] for r < 64 (head 0).
            nc.scalar.dma_start(SEL[:], seld[:])
            nc.vector.memset(RRP[0][:], 0.0)
            nc.vector.memset(RRP[1][:], 0.0)
            # ones + pad columns of VA (cols 64-95 of each pair segment)
            va_seg = VA[:].rearrange("p t (r c) -> p t r c", c=160)
            nc.gpsimd.memset(va_seg[:, :, :, 64:65], 1.0)
            nc.gpsimd.memset(va_seg[:, :, :, 65:96], 0.0)

            # ---- emission helpers ----
            def emit_v(psl, t):
                xtile = XTN[t // 4]
                to = (t % 4) * 128
                for kc in range(CK):
                    nc.tensor.matmul(
                        psl,
                        xtile[:, kc, to:to + 128],
                        WQV[:, kc, :],
                        start=(kc == 0),
                        stop=(kc == CK - 1),
                    )
                src = psl.rearrange("p (r s c) -> p r s c", s=2, c=64)
                dst = VA[:, t, :].rearrange("p (r c) -> p r c", c=160)
                nc.vector.tensor_copy(dst[:, :, 0:64], src[:, :, 0, :])
                nc.vector.tensor_copy(dst[:, :, 96:160], src[:, :, 1, :])

            def emit_vgroup(t2):
                t3 = scp.tile([128, 1024], F32, tag="sc", name="vps")
                emit_v(t3[:, 0:512], t2)
                emit_v(t3[:, 512:1024], t2 + 1)

            def emit_v_half(psl, t, vh):
                # half-width v projection (v dims vh*256..): lets the first
                # matmuls start once only half of x/wv has landed.
                xtile = XTN[t // 4]
                to = (t % 4) * 128
                for kc in range(CK):
                    nc.tensor.matmul(
                        psl,
                        xtile[:, kc, to:to + 128],
                        WQV[:, kc, vh * 256:(vh + 1) * 256],
                        start=(kc == 0),
                        stop=(kc == CK - 1),
                    )
                src = psl.rearrange("p (r s c) -> p r s c", r=2, s=2, c=64)
                dst = VA[:, t, :].rearrange("p (r c) -> p r c", c=160)
                dst = dst[:, 2 * vh:2 * vh + 2, :]
                nc.vector.tensor_copy(dst[:, :, 0:64], src[:, :, 0, :])
                nc.vector.tensor_copy(dst[:, :, 96:160], src[:, :, 1, :])

            def emit_qk(pair, qk, n):
                m = pair + 4 * qk
                psl = scp.tile([128, 1024], F32, tag="sc", name="qkps")[:, 0:512]
                for kc in range(CK):
                    nc.tensor.matmul(
                        psl,
                        WQM[m][:, kc, :],
                        XTN[n][:, kc, :],
                        start=(kc == 0),
                        stop=(kc == CK - 1),
                    )
                ns = slice(n * 512, (n + 1) * 512)
                dst = (QT if qk == 0 else KPP)[pair]
                nc.vector.tensor_scalar_add(dst[:, ns], psl, BQK[:, m:m + 1])

            def emit_yjob(n, mo2):
                t3 = scp.tile([128, 1024], F32, tag="sc", name="yps")
                ns = slice(n * 512, (n + 1) * 512)
                for s in range(2):
                    mo = mo2 + s
                    psl = t3[:, s * 512:(s + 1) * 512]
                    for kc in range(4):
                        nc.tensor.matmul(
                            psl,
                            WOUT[:, kc, mo * 128:(mo + 1) * 128],
                            OT[kc][:, ns],
                            start=(kc == 0),
                            stop=(kc == 3),
                        )
                    ys = yst.tile([128, 512], BF16, tag="ys", name="ys")
                    nc.scalar.copy(ys[:], psl)
                    nc.sync.dma_start(
                        yT[mo * 128:(mo + 1) * 128, ns], ys[:]
                    )

            # ---- normalization ----
            pending_norms = []
            _nrm_ctr = [0]

            def flush_norms():
                while pending_norms:
                    pending_norms.pop(0)()

            def norm_part1(pair, j, oaccj):
                # pull the two sum-of-exp rows out of PSUM, spread both over
                # 64 partitions x 16 with one DMA, reciprocal wide, gather
                # into rows 0 (head 1) / 1 (head 0) of a zero-padded tile.
                rc = nrm.tile([128, 512], F32, tag="rc", name="rc")
                nc.vector.tensor_copy(rc[64:65, :], oaccj[64:65, 0:512])
                nc.vector.tensor_copy(rc[32:33, :], oaccj[32:33, 512:1024])
                # pull the two heads' outputs to SBUF immediately so the
                # single oaccj PSUM buffer frees for the next query chunk
                oc = nrm.tile([128, 512], F32, tag="oc", name="oc")
                nc.vector.tensor_copy(oc[0:64, :], oaccj[0:64, 0:512])
                nc.vector.tensor_copy(oc[64:128, :], oaccj[64:128, 512:1024])
                rs = nrm.tile([64, 16], F32, tag="rs", name="rs")
                nc.sync.dma_start(out=rs[:, 0:8], in_=rc[64:65, :])
                nc.sync.dma_start(out=rs[:, 8:16], in_=rc[32:33, :])
                nc.vector.reciprocal(rs[:], rs[:])
                rsb = nrm.tile([64, 16], BF16, tag="rsb", name="rsb")
                nc.vector.tensor_copy(rsb[:], rs[:])
                rrp = RRP[_nrm_ctr[0] % 2]
                _nrm_ctr[0] += 1
                nc.sync.dma_start(out=rrp[0:1, 0:512], in_=rsb[:, 0:8])
                nc.sync.dma_start(out=rrp[1:2, 0:512], in_=rsb[:, 8:16])
                return oc, rrp

            def norm_part2(pair, j, oc, rrp):
                # replicate 1/sumexp to all partitions with one PE matmul
                # (SEL routes head 0/1 rows), then scale straight into OT.
                js = slice(j * 512, (j + 1) * 512)
                bc = scp.tile([128, 1024], F32, tag="sc", name="bc")
                nc.tensor.matmul(bc[:, 0:512], SEL[:], rrp[:], start=True, stop=True)
                bcv = bcp.tile([128, 512], F32, tag="bcv", name="bcv")
                nc.vector.tensor_copy(bcv[:], bc[:, 0:512])
                nc.vector.tensor_mul(OT[pair][:, js], oc[:], bcv[:])

            # ---- filler schedule (PE work drained into ACT-bound gaps) ----
            fillers = []

            def drain(k=1):
                for _ in range(min(k, len(fillers))):
                    fillers.pop(0)()

            def F_v(t2):
                return lambda: emit_vgroup(t2)

            def F_qk(pair, qk, n):
                return lambda: emit_qk(pair, qk, n)

            def F_y(n, mo2):
                return lambda: emit_yjob(n, mo2)

            SCHED = {
                (0, 0): [F_qk(0, 0, 1), F_qk(0, 1, 1)],
                (0, 1): [F_v(4), F_qk(0, 0, 2), F_v(6), F_qk(0, 1, 2)],
                (0, 2): [F_v(8), F_v(10), F_qk(0, 0, 3), F_qk(0, 1, 3)],
                (0, 3): [F_v(12), F_v(14), F_qk(1, 0, 0), F_qk(1, 1, 0),
                         F_qk(1, 0, 1), F_qk(1, 1, 1)],
                (1, 0): [F_qk(1, 0, 2), F_qk(1, 1, 2)],
                (1, 1): [F_qk(1, 0, 3), F_qk(1, 1, 3)],
                (1, 2): [F_qk(2, 0, 0), F_qk(2, 1, 0), F_qk(2, 0, 1),
                         F_qk(2, 1, 1)],
                (1, 3): [F_qk(2, 0, 2), F_qk(2, 1, 2), F_qk(2, 0, 3),
                         F_qk(2, 1, 3)],
                (2, 0): [],
                (2, 1): [F_qk(3, 0, 0), F_qk(3, 1, 0)],
                (2, 2): [F_qk(3, 0, 1), F_qk(3, 1, 1), F_qk(3, 0, 2),
                         F_qk(3, 1, 2)],
                (2, 3): [F_qk(3, 0, 3), F_qk(3, 1, 3)],
                (3, 0): [],
                (3, 1): [F_y(0, 0), F_y(0, 2), F_y(0, 4), F_y(0, 6)],
                (3, 2): [F_y(1, 0), F_y(1, 2), F_y(1, 4), F_y(1, 6)],
                (3, 3): [F_y(2, 0), F_y(2, 2), F_y(2, 4), F_y(2, 6)],
            }

            # ---- prologue: enough v/qk for (pair 0, j = 0) ----
            t3 = scp.tile([128, 1024], F32, tag="sc", name="vps")
            emit_v_half(t3[:, 0:256], 0, 0)
            emit_v_half(t3[:, 256:512], 0, 1)
            emit_v_half(t3[:, 512:768], 1, 0)
            emit_v_half(t3[:, 768:1024], 1, 1)
            emit_qk(0, 0, 0)
            emit_vgroup(2)
            emit_qk(0, 1, 0)

            # ---- attention ----
            for pair in range(4):
                for j in range(QC):
                    fillers.extend(SCHED[(pair, j)])
                    nb = 4 * (j + 1)
                    js0 = j * 512
                    oaccj = oap.tile([128, 1024], F32, tag="oa", name="oaccj")
                    for k in range(0, nb, 2):
                        scs = []
                        pts = []
                        for i in (k, k + 1):
                            d = i - 4 * j
                            c0 = 128 * max(d, 0)
                            sc = scp.tile([128, 1024], F32, tag="sc", name="sc")
                            nc.tensor.matmul(
                                sc[:, c0:512],
                                KPP[pair][0:64, i * 128:(i + 1) * 128],
                                QT[pair][0:64, js0 + c0:js0 + 512],
                                start=True, stop=True,
                            )
                            nc.tensor.matmul(
                                sc[:, 512 + c0:1024],
                                KPP[pair][64:128, i * 128:(i + 1) * 128],
                                QT[pair][64:128, js0 + c0:js0 + 512],
                                start=True, stop=True,
                            )
                            pt = ptp.tile([128, 1024], BF16, tag="pt")
                            scv = sc.rearrange("p (s c) -> p s c", c=512)
                            ptv = pt.rearrange("p (s c) -> p s c", c=512)
                            nc.scalar.activation(
                                ptv[:, :, c0:512], scv[:, :, c0:512], EXP
                            )
                            if d >= 0:
                                # multiplicative causal mask on the idle
                                # GpSimd engine, off the DVE/ACT queues
                                for s in range(2):
                                    sl = ptv[:, s, c0:c0 + 128]
                                    nc.gpsimd.tensor_mul(sl, sl, MASKD[:])
                            scs.append(sc)
                            pts.append(pt)
                        if k == 2:
                            flush_norms()
                        # pair 3's fillers are out-proj jobs that read OT
                        # slices written by the k==2 norm flush - draining
                        # them at k==0 would bind to stale OT.
                        if pair != 3 or k >= 2:
                            drain(1)
                        for idx, i in enumerate((k, k + 1)):
                            d = i - 4 * j
                            c0 = 128 * max(d, 0)
                            pt = pts[idx]
                            nc.tensor.matmul(
                                oaccj[:, c0:512],
                                VA[:, i, pair * 160:pair * 160 + 128],
                                pt[:, c0:512],
                                start=(i == 0), stop=(i == nb - 1),
                            )
                            nc.tensor.matmul(
                                oaccj[:, 512 + c0:1024],
                                VA[:, i, pair * 160 + 32:pair * 160 + 160],
                                pt[:, 512 + c0:1024],
                                start=(i == 0), stop=(i == nb - 1),
                            )
                    oc, rrp = norm_part1(pair, j, oaccj)
                    pending_norms.append(
                        lambda pair=pair, j=j, oc=oc, rrp=rrp:
                        norm_part2(pair, j, oc, rrp)
                    )

            flush_norms()
            for mo2 in range(0, 8, 2):
                emit_yjob(3, mo2)

    nc.compile()
    return nc


def _make_maskd():
    p = np.arange(128)[:, None]
    f = np.arange(128)[None, :]
    return (p <= f).astype(np.float32).astype(NPBF16)


def _make_seld():
    s = np.zeros((128, 128), np.float32)
    s[0, 0:64] = 1.0     # rrp row 0 = 1/sumexp0 -> bc rows 0-63 (head 0)
    s[1, 64:128] = 1.0   # rrp row 1 = 1/sumexp1 -> bc rows 64-127 (head 1)
    return s.astype(NPBF16)


def kernel(x, w_qkv, b_qkv, w_out, b_out):
    global _CACHE, LAST_RESULT
    x = np.asarray(x, np.float32)
    w_qkv = np.asarray(w_qkv, np.float32)
    b_qkv = np.asarray(b_qkv, np.float32)
    w_out = np.asarray(w_out, np.float32)
    b_out = np.asarray(b_out, np.float32)

    if _CACHE is None:
        _CACHE = _build()
    nc = _CACHE

    maskd = _make_maskd()
    seld = _make_seld()
    in_maps = []
    bias_corr = []
    for core in range(8):
        b = core // 2
        g = core % 2
        sl = slice(g * 512, (g + 1) * 512)
        wq = w_qkv[:, 0:1024][:, sl] * SCALE
        wk = w_qkv[:, 1024:2048][:, sl]
        wv = w_qkv[:, 2048:3072][:, sl]
        wqkv_c = np.ascontiguousarray(
            np.concatenate([wq, wk, wv], axis=1).astype(NPBF16)
        )
        bq = b_qkv[0:1024][sl] * SCALE
        bk = b_qkv[1024:2048][sl]
        bqk_c = np.ascontiguousarray(
            np.concatenate([bq, bk]).reshape(CK, 128).T.astype(np.float32)
        )
        # v-bias folded through the out-projection on the host
        bias_corr.append(w_out[sl, :].T @ b_qkv[2048:3072][sl])
        in_maps.append(
            {
                "xT": np.ascontiguousarray(x[b].T.astype(NPBF16)),
                "wqkv": wqkv_c,
                "bqk": bqk_c,
                "wout": np.ascontiguousarray(w_out[sl, :].astype(NPBF16)),
                "mskd": maskd,
                "seld": seld,
            }
        )

    res = run_bass_kernel_spmd(nc, in_maps, core_ids=list(range(8)), trace=TRACE)
    LAST_RESULT = res

    out = np.empty((B, T, C), np.float32)
    for b in range(B):
        acc = res.results[2 * b]["yT"].astype(np.float32) + res.results[
            2 * b + 1
        ]["yT"].astype(np.float32)
        bias = b_out + bias_corr[2 * b] + bias_corr[2 * b + 1]
        out[b] = acc.T + bias[None, :]
    return out
